# revision 1
# baseline (speedup 1.0000x reference)
"""HGCN (hypergraph conv net) Trainium2 kernel, 8-core SPMD.

Strategy:
  - Graph-aligned node sharding: core c owns graphs [8c, 8c+8). Each graph is
    packed into a fixed GSLOT-row slot so the per-core program is uniform
    (SPMD) while per-core data (indices, one-hot matrices) differs.
  - Reformulation: e = Binv * (H^T h) @ W  (aggregate raw h into hyperedges
    BEFORE the dense transform). Each core aggregates only its own nodes'
    incidence entries (gather sources are core-local), so the only cross-core
    exchange per layer is ReduceScatter(u_partial) + AllGather(e) on the
    small hyperedge-side tensor.
  - Segment sums are computed as one-hot matmuls on the PE: dma_gather pulls
    128 sorted entries' rows onto the 128 partitions, a host-built one-hot
    matrix (with Binv/Dinv degree scaling folded in) is the stationary
    operand, and PSUM accumulates chunks of a 128-hedge/128-node block.
  - Pooling: mean via one-hot matmul, max via DMA-transpose + per-graph-slot
    reduce_max. Each core holds complete graphs so the tiny MLP runs locally
    and the output is assembled on the host from per-core [2, 8] tiles.
  - bf16 data path, f32 accumulation/statistics.
"""

import math
import os

import numpy as np
from ml_dtypes import bfloat16

import concourse.bacc as bacc
import concourse.bass as bass
import concourse.mybir as mybir
import concourse.tile as tile
from concourse.bass import IndirectOffsetOnAxis  # noqa: F401  (kept for reference)
from concourse.bass_utils import run_bass_kernel_spmd

# ---------------------------------------------------------------- constants
NCORES = 8
N_NODES = 50000
N_INC = 300000
N_HE = 10000
NG = 64
IN_C = 768
HID = 512
NL = 3
NCLS = 2

P = 128
GPC = NG // NCORES            # graphs per core
NHB = 80                      # hedge blocks of 128 -> HE_PAD rows
HE_PAD = NHB * P              # 10240
HE_SH = HE_PAD // NCORES      # 1280 rows per core after ReduceScatter
SB_CH = 16                    # gather-batch size in chunks (24*128 idx/batch)
LN_EPS = 1e-5
NEG = -1.0e30

f32 = mybir.dt.float32
bf16 = mybir.dt.bfloat16
i16 = mybir.dt.int16
AF = mybir.ActivationFunctionType
ALU = mybir.AluOpType


# ---------------------------------------------------------------- host prep
def _wrap_idx(idx, nch):
    """dma_gather index layout: idx j -> [j%16, j//16], tiled to 128 parts."""
    cols = nch * 8
    w = np.zeros((16, cols), np.int16)
    w[np.arange(idx.size) % 16, np.arange(idx.size) // 16] = idx.astype(np.int16)
    return np.tile(w, (8, 1))


def _pm(a):
    """[NB*128, F] row-major blocks -> partition-major [128, NB*F]."""
    nb = a.shape[0] // P
    return np.ascontiguousarray(
        a.reshape(nb, P, a.shape[1]).transpose(1, 0, 2).reshape(P, nb * a.shape[1])
    )


def _side_schedule(block_of_entry, n_blocks, per_core_entries):
    """Uniform chunk schedule: chunks per block = max over cores, min 1."""
    counts = np.zeros((NCORES, n_blocks), np.int64)
    for c in range(NCORES):
        blk = block_of_entry[c]
        np.add.at(counts[c], blk, 1)
    chunks = np.maximum(1, -(-counts.max(axis=0) // P))  # ceil
    chunk_map = []  # (block, is_first, is_last)
    for k in range(n_blocks):
        for j in range(chunks[k]):
            chunk_map.append((k, j == 0, j == chunks[k] - 1))
    return chunks, chunk_map


def _batches(n_chunks):
    out = []
    s = 0
    while s < n_chunks:
        n = min(SB_CH, n_chunks - s)
        out.append((s, n))
        s += n
    return out


def preprocess(inputs):
    x = np.asarray(inputs["x"], np.float32)
    node_idx = np.asarray(inputs["node_idx"]).astype(np.int64)
    hedge_idx = np.asarray(inputs["hedge_idx"]).astype(np.int64)
    batch = np.asarray(inputs["batch"]).astype(np.int64)

    cnt_g = np.bincount(batch, minlength=NG)
    gstart = np.concatenate([[0], np.cumsum(cnt_g)])
    gslot = max(896, -(-int(cnt_g.max()) // P) * P)
    npcp = GPC * gslot                      # local (padded) rows per core
    nnb = npcp // P                         # node blocks per core

    rank_in_graph = np.arange(N_NODES) - gstart[batch]
    core_of_node = batch // GPC
    lrow = (batch % GPC) * gslot + rank_in_graph     # local row on its core

    D = np.bincount(node_idx, minlength=N_NODES)
    B = np.bincount(hedge_idx, minlength=N_HE)
    Dinv = np.where(D > 0, 1.0 / np.maximum(D, 1), 0.0).astype(np.float32)
    Binv = np.where(B > 0, 1.0 / np.maximum(B, 1), 0.0).astype(np.float32)

    ecore = core_of_node[node_idx]

    # Dinv in per-core local-row layout
    npcp_tmp = GPC * gslot
    Dinv_l = np.zeros((NCORES, npcp_tmp), np.float32)
    Dinv_l[core_of_node, lrow] = Dinv

    # ---- per-core entry lists
    e_lrow, e_hedge = [], []
    for c in range(NCORES):
        sel = ecore == c
        e_lrow.append(lrow[node_idx[sel]])
        e_hedge.append(hedge_idx[sel])

    # ---- hedge-side schedule (blocks of 128 hedges)
    hblk = [eh // P for eh in e_hedge]
    h_chunks, h_map = _side_schedule(hblk, NHB, e_hedge)
    ch_h = len(h_map)
    chunk_base_h = np.concatenate([[0], np.cumsum(np.maximum(1, h_chunks))])

    # ---- node-side schedule (blocks of 128 local rows)
    nblk = [el // P for el in e_lrow]
    n_chunks, n_map = _side_schedule(nblk, nnb, e_lrow)
    ch_n = len(n_map)
    chunk_base_n = np.concatenate([[0], np.cumsum(np.maximum(1, n_chunks))])

    per_core = []
    for c in range(NCORES):
        # hedge side: gather h rows by local node row, scatter to hedge slot
        gih = np.zeros(ch_h * P, np.int64)
        sh = np.zeros((ch_h, P, P), np.float32)
        order = np.argsort(e_hedge[c], kind="stable")
        eh = e_hedge[c][order]
        el = e_lrow[c][order]
        blk = eh // P
        # position within block region
        for k in range(NHB):
            m = blk == k
            n = int(m.sum())
            if n == 0:
                continue
            base = chunk_base_h[k] * P
            pos = base + np.arange(n)
            gih[pos] = el[m]
            sh[pos // P, pos % P, eh[m] - k * P] = Binv[eh[m]]

        # node side: gather e rows by global hedge row, scatter to node slot
        gin = np.zeros(ch_n * P, np.int64)
        sn = np.zeros((ch_n, P, P), np.float32)
        order = np.argsort(e_lrow[c], kind="stable")
        el = e_lrow[c][order]
        eh = e_hedge[c][order]
        blk = el // P
        for b in range(nnb):
            m = blk == b
            n = int(m.sum())
            if n == 0:
                continue
            base = chunk_base_n[b] * P
            pos = base + np.arange(n)
            gin[pos] = eh[m]
            sn[pos // P, pos % P, el[m] - b * P] = Dinv_l[c, el[m]]

        # x in local layout, tiled per (block, k-chunk): [128, nnb*768]
        xl = np.zeros((npcp, IN_C), np.float32)
        nodes_c = np.nonzero(core_of_node == np.int64(c))[0]
        xl[lrow[nodes_c]] = x[nodes_c]
        nkc = IN_C // P
        xkm = np.ascontiguousarray(
            xl.reshape(nnb, P, nkc, P).transpose(3, 0, 2, 1).reshape(P, nnb * IN_C)
        ).astype(bfloat16)

        # pooling one-hot (mean) and masks
        pp = np.zeros((npcp, GPC), np.float32)
        gmask = np.zeros((P, GPC), np.float32)
        maskcol = np.full((npcp, 1), NEG, np.float32)
        for g in range(GPC):
            gg = c * GPC + g
            n = int(cnt_g[gg])
            if n > 0:
                pp[g * gslot : g * gslot + n, g] = 1.0 / max(n, 1)
                gmask[:, g] = 1.0
                maskcol[g * gslot : g * gslot + n] = 0.0

        per_core.append(
            dict(
                xkm=xkm,
                S_h=_pm(sh.reshape(ch_h * P, P)).astype(bfloat16),
                idx_h=_wrap_idx(gih, ch_h),
                S_n=_pm(sn.reshape(ch_n * P, P)).astype(bfloat16),
                idx_n=_wrap_idx(gin, ch_n),
                P_pm=_pm(pp).astype(bfloat16),
                maskcol_pm=_pm(maskcol),
                gmask=gmask,
            )
        )

    # ---- shared weights
    bcast = lambda v: np.ascontiguousarray(
        np.broadcast_to(np.asarray(v, np.float32), (P, HID))
    )
    shared = dict(
        Win=np.asarray(inputs["W_in"], np.float32).astype(bfloat16),
        Wc=np.asarray(inputs["conv_W"], np.float32)
        .reshape(NL * HID, HID)
        .astype(bfloat16),
        binb=bcast(inputs["b_in"]),
        convb=np.concatenate([bcast(np.asarray(inputs["conv_b"])[i]) for i in range(NL)]),
        lng=np.concatenate([bcast(np.asarray(inputs["ln_g"])[i]) for i in range(NL)]),
        lnb=np.concatenate([bcast(np.asarray(inputs["ln_b"])[i]) for i in range(NL)]),
        Wp0=np.asarray(inputs["W_p0"], np.float32).astype(bfloat16),
        Wp1=np.asarray(inputs["W_p1"], np.float32).astype(bfloat16),
        Wc0=np.asarray(inputs["W_c0"], np.float32).astype(bfloat16),
        Wc1=np.asarray(inputs["W_c1"], np.float32).astype(bfloat16),
        bp0T=np.ascontiguousarray(
            np.asarray(inputs["b_p0"], np.float32).reshape(4, P).T
        ),
        bp1T=np.ascontiguousarray(
            np.asarray(inputs["b_p1"], np.float32).reshape(2, P).T
        ),
        bc0T=np.ascontiguousarray(
            np.asarray(inputs["b_c0"], np.float32).reshape(1, P).T
        ),
        bc1=np.asarray(inputs["b_c1"], np.float32).reshape(NCLS, 1),
    )

    sched = dict(
        gslot=gslot,
        npcp=npcp,
        nnb=nnb,
        ch_h=ch_h,
        h_map=h_map,
        ch_n=ch_n,
        n_map=n_map,
    )
    return sched, shared, per_core


# ---------------------------------------------------------------- builder
def build(sched, n_cores=NCORES):
    stage = int(os.environ.get("KSTAGE", "99"))
    npcp = sched["npcp"]
    nnb = sched["nnb"]
    ch_h = sched["ch_h"]
    ch_n = sched["ch_n"]
    he_sh = HE_PAD // n_cores
    rg = [list(range(n_cores))]

    nc = bacc.Bacc("TRN2", target_bir_lowering=False, debug=False, num_devices=n_cores)

    def inp(name, shape, dt):
        return nc.dram_tensor(name, shape, dt, kind="ExternalInput").ap()

    xkm = inp("xkm", [P, nnb * IN_C], bf16)
    S_h = inp("S_h", [P, ch_h * P], bf16)
    idx_h = inp("idx_h", [P, ch_h * 8], i16)
    S_n = inp("S_n", [P, ch_n * P], bf16)
    idx_n = inp("idx_n", [P, ch_n * 8], i16)
    P_pm = inp("P_pm", [P, nnb * GPC], bf16)
    maskcol_pm = inp("maskcol_pm", [P, nnb], f32)
    gmask = inp("gmask", [P, GPC], f32)
    Win = inp("Win", [IN_C, HID], bf16)
    Wc = inp("Wc", [NL * HID, HID], bf16)
    binb = inp("binb", [P, HID], f32)
    convb = inp("convb", [NL * P, HID], f32)
    lng = inp("lng", [NL * P, HID], f32)
    lnb = inp("lnb", [NL * P, HID], f32)
    Wp0 = inp("Wp0", [2 * HID, HID], bf16)
    Wp1 = inp("Wp1", [HID, HID // 2], bf16)
    Wc0 = inp("Wc0", [HID // 2, HID // 4], bf16)
    Wc1 = inp("Wc1", [HID // 4, NCLS], bf16)
    bp0T = inp("bp0T", [P, 4], f32)
    bp1T = inp("bp1T", [P, 2], f32)
    bc0T = inp("bc0T", [P, 1], f32)
    bc1 = inp("bc1", [NCLS, 1], f32)

    out = nc.dram_tensor("out", [NCLS, GPC], f32, kind="ExternalOutput").ap()

    h0 = nc.dram_tensor("h0", [npcp, HID], bf16).ap()
    hA = nc.dram_tensor("hA", [npcp, HID], bf16).ap()
    hB = nc.dram_tensor("hB", [npcp, HID], bf16).ap()
    u_part = nc.dram_tensor("u_part", [HE_PAD, HID], bf16).ap()
    u_rs = nc.dram_tensor("u_rs", [he_sh, HID], bf16).ap()
    e_loc = nc.dram_tensor("e_loc", [he_sh, HID], bf16).ap()
    e_full = nc.dram_tensor("e_full", [HE_PAD, HID], bf16, addr_space="Shared").ap()

    h_seq = [h0, hA, hB, hA]  # h_in per layer; h_out = h_seq[i+1]

    with tile.TileContext(nc) as tc:
        with (
            tc.tile_pool(name="persist", bufs=1) as pers,
            tc.tile_pool(name="psum", bufs=2, space="PSUM") as pp,
            tc.tile_pool(name="psum_acc", bufs=1, space="PSUM") as pacc,
            tc.tile_pool(name="psum_mlp", bufs=2, space="PSUM") as pmlp,
            tc.tile_pool(name="work", bufs=2) as wk,
            tc.tile_pool(name="wconst", bufs=1) as wkc,
            tc.tile_pool(name="gath", bufs=2) as gp,
            tc.tile_pool(name="stats", bufs=4) as stp,
        ):
            # ---- persistent SBUF: gather indices (reused all layers)
            ixh = pers.tile([P, ch_h * 8], i16, tag="ixh")
            nc.sync.dma_start(out=ixh[:], in_=idx_h[:])
            ixn = pers.tile([P, ch_n * 8], i16, tag="ixn")
            nc.sync.dma_start(out=ixn[:], in_=idx_n[:])
            epst = pers.tile([P, 1], f32, tag="eps")
            nc.vector.memset(epst[:], LN_EPS)

            # ================= input projection =================
            with tc.tile_pool(name="inproj", bufs=1) as ip, tc.tile_pool(
                name="inproj_x", bufs=3
            ) as ipx:
                nkc = IN_C // P
                wts = []
                for kc in range(nkc):
                    t = ip.tile([P, HID], bf16, tag=f"win{kc}")
                    nc.sync.dma_start(out=t[:], in_=Win[kc * P : (kc + 1) * P, :])
                    wts.append(t)
                binb_t = ip.tile([P, HID], f32, tag="binb")
                nc.sync.dma_start(out=binb_t[:], in_=binb[:])

                for b in range(nnb):
                    xt = ipx.tile([P, IN_C], bf16, tag="xkm")
                    nc.sync.dma_start(
                        out=xt[:], in_=xkm[:, b * IN_C : (b + 1) * IN_C]
                    )
                    ps = pp.tile([P, HID], f32, tag="mm")
                    for kc in range(nkc):
                        nc.tensor.matmul(
                            out=ps[:],
                            lhsT=xt[:, kc * P : (kc + 1) * P],
                            rhs=wts[kc][:],
                            start=(kc == 0),
                            stop=(kc == nkc - 1),
                        )
                    t = wk.tile([P, HID], f32, tag="ip_t")
                    nc.vector.tensor_add(t[:], ps[:], binb_t[:])
                    ht = wk.tile([P, HID], bf16, tag="ip_h")
                    nc.scalar.activation(ht[:], t[:], AF.Relu)
                    nc.sync.dma_start(out=h0[b * P : (b + 1) * P, :], in_=ht[:])

            # ================= conv layers =================
            for li in range(NL):
                h_in = h_seq[li]
                h_out = h_seq[li + 1]

                convb_t = wkc.tile([P, HID], f32, tag="convb")
                nc.sync.dma_start(out=convb_t[:], in_=convb[li * P : (li + 1) * P, :])
                lng_t = wkc.tile([P, HID], f32, tag="lng")
                nc.sync.dma_start(out=lng_t[:], in_=lng[li * P : (li + 1) * P, :])
                lnb_t = wkc.tile([P, HID], f32, tag="lnb")
                nc.sync.dma_start(out=lnb_t[:], in_=lnb[li * P : (li + 1) * P, :])

                # ---------- phase A: hedge-side aggregation ----------
                if stage < 2:
                    break
                cur_ps = None
                for (c0, nch) in _batches(ch_h):
                    gt = gp.tile([P, nch * HID], bf16, tag="gt")
                    nc.gpsimd.dma_gather(
                        out_ap=gt[:].rearrange("p (c f) -> p c f", f=HID),
                        in_ap=h_in[:, :],
                        idxs_ap=ixh[:, c0 * 8 : (c0 + nch) * 8],
                        num_idxs=nch * P,
                        num_idxs_reg=nch * P,
                        elem_size=HID,
                        single_packet=False,
                    )
                    st = gp.tile([P, nch * P], bf16, tag="st")
                    nc.sync.dma_start(
                        out=st[:], in_=S_h[:, c0 * P : (c0 + nch) * P]
                    )
                    gt3 = gt[:].rearrange("p (c f) -> p c f", f=HID)
                    for ci in range(nch):
                        k, is_first, is_last = sched["h_map"][c0 + ci]
                        if is_first:
                            cur_ps = pp.tile([P, HID], f32, tag="mm")
                        nc.tensor.matmul(
                            out=cur_ps[:],
                            lhsT=st[:, ci * P : (ci + 1) * P],
                            rhs=gt3[:, ci, :],
                            start=is_first,
                            stop=is_last,
                        )
                        if is_last:
                            ub = wk.tile([P, HID], bf16, tag="u_bf")
                            nc.scalar.copy(ub[:], cur_ps[:])
                            nc.sync.dma_start(
                                out=u_part[k * P : (k + 1) * P, :], in_=ub[:]
                            )

                # ---------- phase B: exchange + transform ----------
                if stage < 3:
                    break
                nc.gpsimd.collective_compute(
                    "ReduceScatter",
                    ALU.add,
                    replica_groups=rg,
                    ins=[u_part[:]],
                    outs=[u_rs[:]],
                )
                if stage < 4:
                    break
                uts = []
                for fc in range(4):
                    t = wkc.tile([P, he_sh], bf16, tag=f"uT{fc}")
                    nc.sync.dma_start(
                        out=t[:], in_=u_rs[:, fc * P : (fc + 1) * P], transpose=True
                    )
                    uts.append(t)
                wcs = []
                for kc in range(4):
                    t = wkc.tile([P, HID], bf16, tag=f"wc{kc}")
                    nc.sync.dma_start(
                        out=t[:], in_=Wc[li * HID + kc * P : li * HID + (kc + 1) * P, :]
                    )
                    wcs.append(t)
                for ht_i in range(he_sh // P):
                    ps = pp.tile([P, HID], f32, tag="mm")
                    for kc in range(4):
                        nc.tensor.matmul(
                            out=ps[:],
                            lhsT=uts[kc][:, ht_i * P : (ht_i + 1) * P],
                            rhs=wcs[kc][:],
                            start=(kc == 0),
                            stop=(kc == 3),
                        )
                    eb = wk.tile([P, HID], bf16, tag="e_bf")
                    nc.scalar.copy(eb[:], ps[:])
                    nc.sync.dma_start(out=e_loc[ht_i * P : (ht_i + 1) * P, :], in_=eb[:])
                if stage < 5:
                    break
                nc.gpsimd.collective_compute(
                    "AllGather",
                    ALU.bypass,
                    replica_groups=rg,
                    ins=[e_loc[:]],
                    outs=[e_full[:]],
                )

                # ---------- phase C: node-side aggregation + LN ----------
                if stage < 6:
                    break
                mask_t = wkc.tile([P, nnb], f32, tag="maskc")
                nc.sync.dma_start(out=mask_t[:], in_=maskcol_pm[:])

                cur_ps = None
                for (c0, nch) in _batches(ch_n):
                    gt = gp.tile([P, nch * HID], bf16, tag="gt")
                    nc.gpsimd.dma_gather(
                        out_ap=gt[:].rearrange("p (c f) -> p c f", f=HID),
                        in_ap=e_full[:, :],
                        idxs_ap=ixn[:, c0 * 8 : (c0 + nch) * 8],
                        num_idxs=nch * P,
                        num_idxs_reg=nch * P,
                        elem_size=HID,
                        single_packet=False,
                    )
                    st = gp.tile([P, nch * P], bf16, tag="st")
                    nc.sync.dma_start(
                        out=st[:], in_=S_n[:, c0 * P : (c0 + nch) * P]
                    )
                    gt3 = gt[:].rearrange("p (c f) -> p c f", f=HID)
                    for ci in range(nch):
                        b, is_first, is_last = sched["n_map"][c0 + ci]
                        if is_first:
                            cur_ps = pp.tile([P, HID], f32, tag="mm")
                        nc.tensor.matmul(
                            out=cur_ps[:],
                            lhsT=st[:, ci * P : (ci + 1) * P],
                            rhs=gt3[:, ci, :],
                            start=is_first,
                            stop=is_last,
                        )
                        if not is_last:
                            continue

                        # ---- post-ops for node block b
                        t = wk.tile([P, HID], f32, tag="pot")
                        nc.vector.tensor_add(t[:], cur_ps[:], convb_t[:])
                        s1 = stp.tile([P, 1], f32, tag="s1")
                        nc.vector.reduce_sum(
                            out=s1[:], in_=t[:], axis=mybir.AxisListType.X
                        )
                        sq = wk.tile([P, HID], bf16, tag="posq")
                        s2 = stp.tile([P, 1], f32, tag="s2")
                        nc.scalar.activation(sq[:], t[:], AF.Square, accum_out=s2[:])
                        mean = stp.tile([P, 1], f32, tag="mean")
                        nc.vector.tensor_scalar_mul(mean[:], s1[:], 1.0 / HID)
                        var = stp.tile([P, 1], f32, tag="var")
                        # var = s2/HID - mean^2
                        nc.vector.tensor_scalar_mul(var[:], s2[:], 1.0 / HID)
                        msq = stp.tile([P, 1], f32, tag="msq")
                        nc.vector.tensor_tensor(
                            out=msq[:], in0=mean[:], in1=mean[:], op=ALU.mult
                        )
                        nc.vector.tensor_tensor(
                            out=var[:], in0=var[:], in1=msq[:], op=ALU.subtract
                        )
                        std = stp.tile([P, 1], f32, tag="std")
                        nc.scalar.activation(std[:], var[:], AF.Sqrt, bias=epst[:, 0:1])
                        rstd = stp.tile([P, 1], f32, tag="rstd")
                        nc.vector.reciprocal(rstd[:], std[:])
                        nmr = stp.tile([P, 1], f32, tag="nmr")
                        nc.vector.tensor_tensor(
                            out=nmr[:], in0=mean[:], in1=rstd[:], op=ALU.mult
                        )
                        nc.vector.tensor_scalar_mul(nmr[:], nmr[:], -1.0)
                        xn = wk.tile([P, HID], f32, tag="poxn")
                        nc.scalar.activation(
                            xn[:], t[:], AF.Identity,
                            bias=nmr[:, 0:1], scale=rstd[:, 0:1],
                        )
                        nc.vector.tensor_tensor(
                            out=xn[:], in0=xn[:], in1=lng_t[:], op=ALU.mult
                        )
                        nc.vector.tensor_tensor(
                            out=xn[:], in0=xn[:], in1=lnb_t[:], op=ALU.add
                        )
                        r = wk.tile([P, HID], bf16, tag="por")
                        nc.vector.tensor_scalar_max(r[:], xn[:], 0.0)
                        if li >= 1:
                            hres = wk.tile([P, HID], bf16, tag="pores")
                            nc.sync.dma_start(
                                out=hres[:], in_=h_in[b * P : (b + 1) * P, :]
                            )
                            nc.vector.tensor_add(r[:], r[:], hres[:])
                        if li == NL - 1:
                            h0t = wk.tile([P, HID], bf16, tag="poh0")
                            nc.sync.dma_start(
                                out=h0t[:], in_=h0[b * P : (b + 1) * P, :]
                            )
                            nc.vector.tensor_add(r[:], r[:], h0t[:])
                            nc.vector.tensor_scalar_add(
                                r[:], r[:], mask_t[:, b : b + 1]
                            )
                        nc.sync.dma_start(
                            out=h_out[b * P : (b + 1) * P, :], in_=r[:]
                        )

            # ================= pooling + MLP =================
            h_fin = h_seq[NL]
            if stage < 7:
                with tc.tile_pool(name="stub", bufs=1) as stub:
                    zt = stub.tile([NCLS, GPC], f32, tag="zt")
                    nc.vector.memset(zt[:], 0.0)
                    nc.sync.dma_start(out=out[:], in_=zt[:])
            else:
              with tc.tile_pool(name="pool", bufs=2) as plp:
                # mean: one-hot matmul, PSUM accumulate over all blocks
                ppool_t = plp.tile([P, nnb * GPC], bf16, tag="Ppm")
                nc.sync.dma_start(out=ppool_t[:], in_=P_pm[:])
                mean_ps = [
                    pacc.tile([P, GPC], f32, tag=f"mean_ps{fc}", name=f"mean_ps{fc}") for fc in range(4)
                ]
                for b in range(nnb):
                    hb = plp.tile([P, HID], bf16, tag="hfin")
                    nc.sync.dma_start(out=hb[:], in_=h_fin[b * P : (b + 1) * P, :])
                    for fc in range(4):
                        nc.tensor.matmul(
                            out=mean_ps[fc][:],
                            lhsT=hb[:, fc * P : (fc + 1) * P],
                            rhs=ppool_t[:, b * GPC : (b + 1) * GPC],
                            start=(b == 0),
                            stop=(b == nnb - 1),
                        )
                # max: transpose loads + per-graph-slot reduce
                gmask_t = plp.tile([P, GPC], f32, tag="gmask")
                nc.sync.dma_start(out=gmask_t[:], in_=gmask[:])

                gkt = []  # 8 k-tiles of gT: 4 mean + 4 max, each [P, GPC] bf16
                for fc in range(4):
                    t = plp.tile([P, GPC], bf16, tag=f"gmean{fc}")
                    nc.vector.tensor_copy(out=t[:], in_=mean_ps[fc][:])
                    gkt.append(t)
                gslot = sched["gslot"]
                for fc in range(4):
                    hT = plp.tile([P, npcp], bf16, tag="hT")
                    nc.sync.dma_start(
                        out=hT[:], in_=h_fin[:, fc * P : (fc + 1) * P], transpose=True
                    )
                    mx = plp.tile([P, GPC], f32, tag=f"mx{fc}")
                    for g in range(GPC):
                        nc.vector.reduce_max(
                            out=mx[:, g : g + 1],
                            in_=hT[:, g * gslot : (g + 1) * gslot],
                            axis=mybir.AxisListType.X,
                        )
                    mxm = plp.tile([P, GPC], bf16, tag=f"mxm{fc}")
                    nc.vector.tensor_tensor(
                        out=mxm[:], in0=mx[:], in1=gmask_t[:], op=ALU.mult
                    )
                    gkt.append(mxm)

                # MLP (transposed): a0T = relu(Wp0^T gT + b)
                wp0t = []
                for kt in range(8):
                    t = plp.tile([P, HID], bf16, tag=f"wp0_{kt}")
                    nc.sync.dma_start(out=t[:], in_=Wp0[kt * P : (kt + 1) * P, :])
                    wp0t.append(t)
                bp0_t = plp.tile([P, 4], f32, tag="bp0")
                nc.sync.dma_start(out=bp0_t[:], in_=bp0T[:])
                a0 = []
                for mt in range(4):
                    ps = pmlp.tile([P, GPC], f32, tag="mlp")
                    for kt in range(8):
                        nc.tensor.matmul(
                            out=ps[:],
                            lhsT=wp0t[kt][:, mt * P : (mt + 1) * P],
                            rhs=gkt[kt][:],
                            start=(kt == 0),
                            stop=(kt == 7),
                        )
                    t = plp.tile([P, GPC], bf16, tag=f"a0_{mt}")
                    nc.scalar.activation(
                        t[:], ps[:], AF.Relu, bias=bp0_t[:, mt : mt + 1]
                    )
                    a0.append(t)
                wp1t = []
                for kt in range(4):
                    t = plp.tile([P, HID // 2], bf16, tag=f"wp1_{kt}")
                    nc.sync.dma_start(out=t[:], in_=Wp1[kt * P : (kt + 1) * P, :])
                    wp1t.append(t)
                bp1_t = plp.tile([P, 2], f32, tag="bp1")
                nc.sync.dma_start(out=bp1_t[:], in_=bp1T[:])
                a1 = []
                for mt in range(2):
                    ps = pmlp.tile([P, GPC], f32, tag="mlp")
                    for kt in range(4):
                        nc.tensor.matmul(
                            out=ps[:],
                            lhsT=wp1t[kt][:, mt * P : (mt + 1) * P],
                            rhs=a0[kt][:],
                            start=(kt == 0),
                            stop=(kt == 3),
                        )
                    t = plp.tile([P, GPC], bf16, tag=f"a1_{mt}")
                    nc.scalar.activation(
                        t[:], ps[:], AF.Identity, bias=bp1_t[:, mt : mt + 1]
                    )
                    a1.append(t)
                wc0t = []
                for kt in range(2):
                    t = plp.tile([P, HID // 4], bf16, tag=f"wc0_{kt}")
                    nc.sync.dma_start(out=t[:], in_=Wc0[kt * P : (kt + 1) * P, :])
                    wc0t.append(t)
                bc0_t = plp.tile([P, 1], f32, tag="bc0")
                nc.sync.dma_start(out=bc0_t[:], in_=bc0T[:])
                ps = pmlp.tile([P, GPC], f32, tag="mlp")
                for kt in range(2):
                    nc.tensor.matmul(
                        out=ps[:],
                        lhsT=wc0t[kt][:],
                        rhs=a1[kt][:],
                        start=(kt == 0),
                        stop=(kt == 1),
                    )
                a2 = plp.tile([P, GPC], bf16, tag="a2")
                nc.scalar.activation(a2[:], ps[:], AF.Relu, bias=bc0_t[:, 0:1])
                wc1t = plp.tile([P, NCLS], bf16, tag="wc1")
                nc.sync.dma_start(out=wc1t[:], in_=Wc1[:])
                bc1_t = plp.tile([NCLS, 1], f32, tag="bc1")
                nc.sync.dma_start(out=bc1_t[:], in_=bc1[:])
                ps2 = pmlp.tile([NCLS, GPC], f32, tag="mlp")
                nc.tensor.matmul(
                    out=ps2[:], lhsT=wc1t[:], rhs=a2[:], start=True, stop=True
                )
                ot = plp.tile([NCLS, GPC], f32, tag="ot")
                nc.scalar.activation(ot[:], ps2[:], AF.Identity, bias=bc1_t[:, 0:1])
                nc.sync.dma_start(out=out[:], in_=ot[:])

    nc.compile()
    return nc


def make_in_maps(shared, per_core):
    maps = []
    for c in range(len(per_core)):
        m = dict(shared)
        m.update(per_core[c])
        m = {k: np.ascontiguousarray(v) for k, v in m.items()}
        maps.append(m)
    return maps


def kernel(**inputs) -> np.ndarray:
    sched, shared, per_core = preprocess(inputs)
    nc = build(sched, NCORES)
    in_maps = make_in_maps(shared, per_core)
    res = run_bass_kernel_spmd(nc, in_maps, list(range(NCORES)))
    full = np.zeros((NG, NCLS), np.float32)
    for c in range(NCORES):
        full[c * GPC : (c + 1) * GPC, :] = res.results[c]["out"].T
    return full



# revision 5
# speedup vs baseline: 1.1259x; 1.1259x over previous
"""HGCN (hypergraph conv net) Trainium2 kernel, 8-core SPMD.

Strategy (v2 — fp8 data path for all gathered traffic):
  - Graph-aligned node sharding: core c owns graphs [8c, 8c+8), packed into
    fixed 896-row slots so the per-core program is uniform (SPMD).
  - Reformulation: e = Binv * (H^T h) @ W. Per layer the only cross-core
    exchange is ReduceScatter(u_partial, bf16) + AllGather(e, fp8).
  - Segment sums are one-hot matmuls on the PE. The one-hot S matrices are
    0/1-valued fp8 (exact); the Binv scaling is applied as a per-partition
    scale at the transform output, and the Dinv scaling is absorbed by the
    layer norm's scale invariance (conv_b == 0 fast path) or applied as a
    per-partition activation scale otherwise.
  - All gather traffic (h rows for the hedge side, e rows for the node side,
    x for the input projection) is fp8e4m3: halves the dominant DMA cost.
  - h (bf16) lives in SBUF across layers: residual adds are done in place on
    the SBUF-resident h, mean pooling reads SBUF; only the fp8 gather copies
    and the final h (for the max-pool transpose) touch DRAM.
"""

import math
import os

import numpy as np
from ml_dtypes import bfloat16, float8_e4m3

import concourse.bacc as bacc
import concourse.bass as bass
import concourse.mybir as mybir
import concourse.tile as tile
from concourse.bass_utils import run_bass_kernel_spmd

# ---------------------------------------------------------------- constants
NCORES = 8
N_NODES = 50000
N_INC = 300000
N_HE = 10000
NG = 64
IN_C = 768
HID = 512
NL = 3
NCLS = 2

P = 128
GPC = NG // NCORES            # graphs per core
NHB = 80                      # hedge blocks of 128 -> HE_PAD rows
HE_PAD = NHB * P              # 10240
HE_SH = HE_PAD // NCORES      # 1280 rows per core after ReduceScatter
SB_CH = 16                    # gather-batch size in chunks (16*128 idx/batch)
LN_EPS = 1e-5
NEG = -1.0e30

f32 = mybir.dt.float32
bf16 = mybir.dt.bfloat16
f8 = mybir.dt.float8e4
i16 = mybir.dt.int16
AF = mybir.ActivationFunctionType
ALU = mybir.AluOpType


# ---------------------------------------------------------------- host prep
def _wrap_idx(idx, nch):
    """dma_gather index layout: idx j -> [j%16, j//16], tiled to 128 parts."""
    cols = nch * 8
    w = np.zeros((16, cols), np.int16)
    w[np.arange(idx.size) % 16, np.arange(idx.size) // 16] = idx.astype(np.int16)
    return np.tile(w, (8, 1))


def _pm(a):
    """[NB*128, F] row-major blocks -> partition-major [128, NB*F]."""
    nb = a.shape[0] // P
    return np.ascontiguousarray(
        a.reshape(nb, P, a.shape[1]).transpose(1, 0, 2).reshape(P, nb * a.shape[1])
    )


def _side_schedule(block_of_entry, n_blocks, per_core_entries):
    """Uniform chunk schedule: chunks per block = max over cores, min 1."""
    counts = np.zeros((NCORES, n_blocks), np.int64)
    for c in range(NCORES):
        blk = block_of_entry[c]
        np.add.at(counts[c], blk, 1)
    chunks = np.maximum(1, -(-counts.max(axis=0) // P))  # ceil
    chunk_map = []  # (block, is_first, is_last)
    for k in range(n_blocks):
        for j in range(chunks[k]):
            chunk_map.append((k, j == 0, j == chunks[k] - 1))
    return chunks, chunk_map


def _batches(n_chunks):
    out = []
    s = 0
    while s < n_chunks:
        n = min(SB_CH, n_chunks - s)
        out.append((s, n))
        s += n
    return out


def preprocess(inputs):
    x = np.asarray(inputs["x"], np.float32)
    node_idx = np.asarray(inputs["node_idx"]).astype(np.int64)
    hedge_idx = np.asarray(inputs["hedge_idx"]).astype(np.int64)
    batch = np.asarray(inputs["batch"]).astype(np.int64)

    cnt_g = np.bincount(batch, minlength=NG)
    gstart = np.concatenate([[0], np.cumsum(cnt_g)])
    gslot = max(896, -(-int(cnt_g.max()) // P) * P)
    npcp = GPC * gslot                      # local (padded) rows per core
    nnb = npcp // P                         # node blocks per core

    rank_in_graph = np.arange(N_NODES) - gstart[batch]
    core_of_node = batch // GPC
    lrow = (batch % GPC) * gslot + rank_in_graph     # local row on its core

    D = np.bincount(node_idx, minlength=N_NODES)
    B = np.bincount(hedge_idx, minlength=N_HE)
    Dinv = np.where(D > 0, 1.0 / np.maximum(D, 1), 0.0).astype(np.float32)
    Binv = np.where(B > 0, 1.0 / np.maximum(B, 1), 0.0).astype(np.float32)

    ecore = core_of_node[node_idx]

    # Dinv in per-core local-row layout
    Dinv_l = np.zeros((NCORES, npcp), np.float32)
    Dinv_l[core_of_node, lrow] = Dinv

    # Binv padded to HE_PAD
    Binv_p = np.zeros((HE_PAD,), np.float32)
    Binv_p[:N_HE] = Binv

    # ---- per-core entry lists
    e_lrow, e_hedge = [], []
    for c in range(NCORES):
        sel = ecore == c
        e_lrow.append(lrow[node_idx[sel]])
        e_hedge.append(hedge_idx[sel])

    # ---- hedge-side schedule (blocks of 128 hedges)
    hblk = [eh // P for eh in e_hedge]
    h_chunks, h_map = _side_schedule(hblk, NHB, e_hedge)
    ch_h = len(h_map)
    chunk_base_h = np.concatenate([[0], np.cumsum(np.maximum(1, h_chunks))])

    # ---- node-side schedule (blocks of 128 local rows)
    nblk = [el // P for el in e_lrow]
    n_chunks, n_map = _side_schedule(nblk, nnb, e_lrow)
    ch_n = len(n_map)
    chunk_base_n = np.concatenate([[0], np.cumsum(np.maximum(1, n_chunks))])

    per_core = []
    for c in range(NCORES):
        # hedge side: gather h rows by local node row, scatter to hedge slot.
        # S entries are 1.0 (Binv applied later as per-partition scale).
        gih = np.zeros(ch_h * P, np.int64)
        sh = np.zeros((ch_h, P, P), np.float32)
        order = np.argsort(e_hedge[c], kind="stable")
        eh = e_hedge[c][order]
        el = e_lrow[c][order]
        blk = eh // P
        for k in range(NHB):
            m = blk == k
            n = int(m.sum())
            if n == 0:
                continue
            base = chunk_base_h[k] * P
            pos = base + np.arange(n)
            gih[pos] = el[m]
            sh[pos // P, pos % P, eh[m] - k * P] = 1.0

        # node side: gather e rows by global hedge row, scatter to node slot.
        # S entries are 1.0 (Dinv absorbed by LN scale invariance).
        gin = np.zeros(ch_n * P, np.int64)
        sn = np.zeros((ch_n, P, P), np.float32)
        order = np.argsort(e_lrow[c], kind="stable")
        el = e_lrow[c][order]
        eh = e_hedge[c][order]
        blk = el // P
        for b in range(nnb):
            m = blk == b
            n = int(m.sum())
            if n == 0:
                continue
            base = chunk_base_n[b] * P
            pos = base + np.arange(n)
            gin[pos] = eh[m]
            sn[pos // P, pos % P, el[m] - b * P] = 1.0

        # x in local layout, tiled per (block, k-chunk): [128, nnb*768] fp8
        xl = np.zeros((npcp, IN_C), np.float32)
        nodes_c = np.nonzero(core_of_node == np.int64(c))[0]
        xl[lrow[nodes_c]] = x[nodes_c]
        nkc = IN_C // P
        xkm = np.ascontiguousarray(
            xl.reshape(nnb, P, nkc, P).transpose(3, 0, 2, 1).reshape(P, nnb * IN_C)
        ).astype(float8_e4m3)

        # Binv for this core's ReduceScatter shard, per-partition column layout
        bsh = Binv_p[c * HE_SH : (c + 1) * HE_SH]
        binv_sh = np.ascontiguousarray(bsh.reshape(HE_SH // P, P).T).astype(np.float32)

        # Dinv in per-partition column layout [128, nnb]
        dinv_col = np.ascontiguousarray(
            Dinv_l[c].reshape(nnb, P).T
        ).astype(np.float32)

        # pooling one-hot (mean) and masks
        pp = np.zeros((npcp, GPC), np.float32)
        gmask = np.zeros((P, GPC), np.float32)
        maskcol = np.full((npcp, 1), NEG, np.float32)
        for g in range(GPC):
            gg = c * GPC + g
            n = int(cnt_g[gg])
            if n > 0:
                pp[g * gslot : g * gslot + n, g] = 1.0 / max(n, 1)
                gmask[:, g] = 1.0
                maskcol[g * gslot : g * gslot + n] = 0.0

        per_core.append(
            dict(
                xkm=xkm,
                S_h=_pm(sh.reshape(ch_h * P, P)).astype(float8_e4m3),
                idx_h=_wrap_idx(gih, ch_h),
                S_n=_pm(sn.reshape(ch_n * P, P)).astype(float8_e4m3),
                idx_n=_wrap_idx(gin, ch_n),
                binv_sh=binv_sh,
                dinv_col=dinv_col,
                P_pm=_pm(pp).astype(bfloat16),
                maskcol_pm=_pm(maskcol),
                gmask=gmask,
            )
        )

    # ---- shared weights
    bcast = lambda v: np.ascontiguousarray(
        np.broadcast_to(np.asarray(v, np.float32), (P, HID))
    )
    shared = dict(
        Win=np.asarray(inputs["W_in"], np.float32).astype(bfloat16),
        Wc=np.asarray(inputs["conv_W"], np.float32)
        .reshape(NL * HID, HID)
        .astype(bfloat16),
        binb=bcast(inputs["b_in"]),
        convb=np.concatenate([bcast(np.asarray(inputs["conv_b"])[i]) for i in range(NL)]),
        lng=np.concatenate([bcast(np.asarray(inputs["ln_g"])[i]) for i in range(NL)]),
        lnb=np.concatenate([bcast(np.asarray(inputs["ln_b"])[i]) for i in range(NL)]),
        Wp0=np.asarray(inputs["W_p0"], np.float32).astype(bfloat16),
        Wp1=np.asarray(inputs["W_p1"], np.float32).astype(bfloat16),
        Wc0=np.asarray(inputs["W_c0"], np.float32).astype(bfloat16),
        Wc1=np.asarray(inputs["W_c1"], np.float32).astype(bfloat16),
        bp0T=np.ascontiguousarray(
            np.asarray(inputs["b_p0"], np.float32).reshape(4, P).T
        ),
        bp1T=np.ascontiguousarray(
            np.asarray(inputs["b_p1"], np.float32).reshape(2, P).T
        ),
        bc0T=np.ascontiguousarray(
            np.asarray(inputs["b_c0"], np.float32).reshape(1, P).T
        ),
        bc1=np.asarray(inputs["b_c1"], np.float32).reshape(NCLS, 1),
    )

    # zero-bias / unit-gain fast paths (checked against the actual inputs)
    flags = dict(
        bin_zero=bool(np.all(np.asarray(inputs["b_in"]) == 0)),
        convb_zero=bool(np.all(np.asarray(inputs["conv_b"]) == 0)),
        lng_one=bool(np.all(np.asarray(inputs["ln_g"]) == 1)),
        lnb_zero=bool(np.all(np.asarray(inputs["ln_b"]) == 0)),
    )

    sched = dict(
        gslot=gslot,
        npcp=npcp,
        nnb=nnb,
        ch_h=ch_h,
        h_map=h_map,
        ch_n=ch_n,
        n_map=n_map,
        flags=flags,
    )
    return sched, shared, per_core


# ---------------------------------------------------------------- builder
def build(sched, n_cores=NCORES):
    stage = int(os.environ.get("KSTAGE", "99"))
    npcp = sched["npcp"]
    nnb = sched["nnb"]
    ch_h = sched["ch_h"]
    ch_n = sched["ch_n"]
    fl = sched["flags"]
    he_sh = HE_PAD // n_cores
    nhb_sh = he_sh // P
    rg = [list(range(n_cores))]

    nc = bacc.Bacc("TRN2", target_bir_lowering=False, debug=False, num_devices=n_cores)

    def inp(name, shape, dt):
        return nc.dram_tensor(name, shape, dt, kind="ExternalInput").ap()

    xkm = inp("xkm", [P, nnb * IN_C], f8)
    S_h = inp("S_h", [P, ch_h * P], f8)
    idx_h = inp("idx_h", [P, ch_h * 8], i16)
    S_n = inp("S_n", [P, ch_n * P], f8)
    idx_n = inp("idx_n", [P, ch_n * 8], i16)
    binv_sh = inp("binv_sh", [P, nhb_sh], f32)
    dinv_col = inp("dinv_col", [P, nnb], f32)
    P_pm = inp("P_pm", [P, nnb * GPC], bf16)
    maskcol_pm = inp("maskcol_pm", [P, nnb], f32)
    gmask = inp("gmask", [P, GPC], f32)
    Win = inp("Win", [IN_C, HID], bf16)
    Wc = inp("Wc", [NL * HID, HID], bf16)
    binb = inp("binb", [P, HID], f32)
    convb = inp("convb", [NL * P, HID], f32)
    lng = inp("lng", [NL * P, HID], f32)
    lnb = inp("lnb", [NL * P, HID], f32)
    Wp0 = inp("Wp0", [2 * HID, HID], bf16)
    Wp1 = inp("Wp1", [HID, HID // 2], bf16)
    Wc0 = inp("Wc0", [HID // 2, HID // 4], bf16)
    Wc1 = inp("Wc1", [HID // 4, NCLS], bf16)
    bp0T = inp("bp0T", [P, 4], f32)
    bp1T = inp("bp1T", [P, 2], f32)
    bc0T = inp("bc0T", [P, 1], f32)
    bc1 = inp("bc1", [NCLS, 1], f32)

    out = nc.dram_tensor("out", [NCLS, GPC], f32, kind="ExternalOutput").ap()

    # fp8 gather sources (one per layer input) + final h for max-pool transpose
    h8 = [nc.dram_tensor(f"h8_{i}", [npcp, HID], f8).ap() for i in range(NL)]
    h_fin = nc.dram_tensor("h_fin", [npcp, HID], bf16).ap()
    u_part = nc.dram_tensor("u_part", [HE_PAD, HID], bf16).ap()
    u_rs = nc.dram_tensor("u_rs", [he_sh, HID], bf16).ap()
    e_loc = nc.dram_tensor("e_loc", [he_sh, HID], f8).ap()
    e_full = nc.dram_tensor("e_full", [HE_PAD, HID], f8, addr_space="Shared").ap()

    with tile.TileContext(nc) as tc:
        with (
            tc.tile_pool(name="persist", bufs=1) as pers,
            tc.tile_pool(name="psum", bufs=2, space="PSUM") as pp,
            tc.tile_pool(name="psum_acc", bufs=1, space="PSUM") as pacc,
            tc.tile_pool(name="psum_mlp", bufs=2, space="PSUM") as pmlp,
            tc.tile_pool(name="work", bufs=2) as wk,
            tc.tile_pool(name="wconst", bufs=1) as wkc,
            tc.tile_pool(name="stats", bufs=4) as stp,
        ):
            # ---- persistent SBUF: h tiles, scales
            epst = pers.tile([P, 1], f32, tag="eps")
            nc.vector.memset(epst[:], LN_EPS)
            binv_t = pers.tile([P, nhb_sh], f32, tag="binv")
            nc.sync.dma_start(out=binv_t[:], in_=binv_sh[:])
            dinv_t = pers.tile([P, nnb], f32, tag="dinv")
            nc.sync.dma_start(out=dinv_t[:], in_=dinv_col[:])
            # h in SBUF across layers: h0 (global residual) + current h,
            # updated in place by the per-layer residual add
            h0_sb = pers.tile([P, nnb * HID], bf16, tag="h0sb", name="h0sb")
            h_cur = pers.tile([P, nnb * HID], bf16, tag="hcur", name="hcur")

            # ================= input projection =================
            with tc.tile_pool(name="inproj", bufs=1) as ip, tc.tile_pool(
                name="inproj_x", bufs=3
            ) as ipx:
                nkc = IN_C // P
                wts = []
                for kc in range(nkc):
                    t = ip.tile([P, HID], bf16, tag=f"win{kc}")
                    nc.sync.dma_start(out=t[:], in_=Win[kc * P : (kc + 1) * P, :])
                    wts.append(t)
                if not fl["bin_zero"]:
                    binb_t = ip.tile([P, HID], f32, tag="binb")
                    nc.sync.dma_start(out=binb_t[:], in_=binb[:])

                for b in range(nnb):
                    xt = ipx.tile([P, IN_C], f8, tag="xkm")
                    nc.sync.dma_start(
                        out=xt[:], in_=xkm[:, b * IN_C : (b + 1) * IN_C]
                    )
                    ps = pp.tile([P, HID], f32, tag="mm")
                    for kc in range(nkc):
                        nc.tensor.matmul(
                            out=ps[:],
                            lhsT=xt[:, kc * P : (kc + 1) * P],
                            rhs=wts[kc][:],
                            start=(kc == 0),
                            stop=(kc == nkc - 1),
                        )
                    src = ps
                    if not fl["bin_zero"]:
                        t = wk.tile([P, HID], f32, tag="ip_t")
                        nc.vector.tensor_add(t[:], ps[:], binb_t[:])
                        src = t
                    nc.scalar.activation(
                        h0_sb[:, b * HID : (b + 1) * HID], src[:], AF.Relu
                    )
                    h8t = wk.tile([P, HID], f8, tag="ip_h8")
                    nc.scalar.activation(h8t[:], src[:], AF.Relu)
                    nc.sync.dma_start(out=h8[0][b * P : (b + 1) * P, :], in_=h8t[:])

            # ================= conv layers =================
            with tc.tile_pool(name="convidx", bufs=1) as cvi, tc.tile_pool(
                name="gath", bufs=2
            ) as gp:
              ixh = cvi.tile([P, ch_h * 8], i16, tag="ixh")
              nc.sync.dma_start(out=ixh[:], in_=idx_h[:])
              ixn = cvi.tile([P, ch_n * 8], i16, tag="ixn")
              nc.sync.dma_start(out=ixn[:], in_=idx_n[:])

              for li in range(NL):
                if not fl["convb_zero"]:
                    convb_t = wkc.tile([P, HID], f32, tag="convb")
                    nc.sync.dma_start(
                        out=convb_t[:], in_=convb[li * P : (li + 1) * P, :]
                    )
                if not fl["lng_one"]:
                    lng_t = wkc.tile([P, HID], f32, tag="lng")
                    nc.sync.dma_start(out=lng_t[:], in_=lng[li * P : (li + 1) * P, :])
                if not fl["lnb_zero"]:
                    lnb_t = wkc.tile([P, HID], f32, tag="lnb")
                    nc.sync.dma_start(out=lnb_t[:], in_=lnb[li * P : (li + 1) * P, :])

                # ---------- phase A: hedge-side aggregation ----------
                if stage < 2:
                    break
                cur_ps = None
                for (c0, nch) in _batches(ch_h):
                    gt = gp.tile([P, SB_CH * HID], f8, tag="gt")
                    nc.gpsimd.dma_gather(
                        out_ap=gt[:, : nch * HID].rearrange(
                            "p (c f) -> p c f", f=HID
                        ),
                        in_ap=h8[li][:, :],
                        idxs_ap=ixh[:, c0 * 8 : (c0 + nch) * 8],
                        num_idxs=nch * P,
                        num_idxs_reg=nch * P,
                        elem_size=HID,
                        single_packet=False,
                    )
                    st = gp.tile([P, SB_CH * P], f8, tag="st")
                    nc.sync.dma_start(
                        out=st[:, : nch * P], in_=S_h[:, c0 * P : (c0 + nch) * P]
                    )
                    gt3 = gt[:, : nch * HID].rearrange("p (c f) -> p c f", f=HID)
                    for ci in range(nch):
                        k, is_first, is_last = sched["h_map"][c0 + ci]
                        if is_first:
                            cur_ps = pp.tile([P, HID], f32, tag="mm")
                        nc.tensor.matmul(
                            out=cur_ps[:],
                            lhsT=st[:, ci * P : (ci + 1) * P],
                            rhs=gt3[:, ci, :],
                            start=is_first,
                            stop=is_last,
                        )
                        if is_last:
                            ub = wk.tile([P, HID], bf16, tag="u_bf")
                            nc.scalar.copy(ub[:], cur_ps[:])
                            nc.sync.dma_start(
                                out=u_part[k * P : (k + 1) * P, :], in_=ub[:]
                            )

                # ---------- phase B: exchange + transform ----------
                if stage < 3:
                    break
                nc.gpsimd.collective_compute(
                    "ReduceScatter",
                    ALU.add,
                    replica_groups=rg,
                    ins=[u_part[:]],
                    outs=[u_rs[:]],
                )
                if stage < 4:
                    break
                uts = []
                for fc in range(4):
                    t = wkc.tile([P, he_sh], bf16, tag=f"uT{fc}")
                    nc.sync.dma_start(
                        out=t[:], in_=u_rs[:, fc * P : (fc + 1) * P], transpose=True
                    )
                    uts.append(t)
                wcs = []
                for kc in range(4):
                    t = wkc.tile([P, HID], bf16, tag=f"wc{kc}")
                    nc.sync.dma_start(
                        out=t[:], in_=Wc[li * HID + kc * P : li * HID + (kc + 1) * P, :]
                    )
                    wcs.append(t)
                for ht_i in range(nhb_sh):
                    ps = pp.tile([P, HID], f32, tag="mm")
                    for kc in range(4):
                        nc.tensor.matmul(
                            out=ps[:],
                            lhsT=uts[kc][:, ht_i * P : (ht_i + 1) * P],
                            rhs=wcs[kc][:],
                            start=(kc == 0),
                            stop=(kc == 3),
                        )
                    eb = wk.tile([P, HID], f8, tag="e_f8")
                    nc.scalar.activation(
                        eb[:], ps[:], AF.Identity,
                        scale=binv_t[:, ht_i : ht_i + 1],
                    )
                    nc.sync.dma_start(out=e_loc[ht_i * P : (ht_i + 1) * P, :], in_=eb[:])
                if stage < 5:
                    break
                nc.gpsimd.collective_compute(
                    "AllGather",
                    ALU.bypass,
                    replica_groups=rg,
                    ins=[e_loc[:]],
                    outs=[e_full[:]],
                )

                # ---------- phase C: node-side aggregation + LN ----------
                if stage < 6:
                    break
                mask_t = wkc.tile([P, nnb], f32, tag="maskc")
                nc.sync.dma_start(out=mask_t[:], in_=maskcol_pm[:])

                cur_ps = None
                for (c0, nch) in _batches(ch_n):
                    gt = gp.tile([P, SB_CH * HID], f8, tag="gt")
                    nc.gpsimd.dma_gather(
                        out_ap=gt[:, : nch * HID].rearrange(
                            "p (c f) -> p c f", f=HID
                        ),
                        in_ap=e_full[:, :],
                        idxs_ap=ixn[:, c0 * 8 : (c0 + nch) * 8],
                        num_idxs=nch * P,
                        num_idxs_reg=nch * P,
                        elem_size=HID,
                        single_packet=False,
                    )
                    st = gp.tile([P, SB_CH * P], f8, tag="st")
                    nc.sync.dma_start(
                        out=st[:, : nch * P], in_=S_n[:, c0 * P : (c0 + nch) * P]
                    )
                    gt3 = gt[:, : nch * HID].rearrange("p (c f) -> p c f", f=HID)
                    for ci in range(nch):
                        b, is_first, is_last = sched["n_map"][c0 + ci]
                        if is_first:
                            cur_ps = pp.tile([P, HID], f32, tag="mm")
                        nc.tensor.matmul(
                            out=cur_ps[:],
                            lhsT=st[:, ci * P : (ci + 1) * P],
                            rhs=gt3[:, ci, :],
                            start=is_first,
                            stop=is_last,
                        )
                        if not is_last:
                            continue

                        # ---- post-ops for node block b
                        # conv_b == 0 fast path: Dinv row scaling is absorbed
                        # by LN's scale invariance, stats read PSUM directly.
                        if fl["convb_zero"]:
                            src = cur_ps
                        else:
                            t = wk.tile([P, HID], f32, tag="pot")
                            nc.scalar.activation(
                                t[:], cur_ps[:], AF.Identity,
                                scale=dinv_t[:, b : b + 1],
                            )
                            nc.vector.tensor_add(t[:], t[:], convb_t[:])
                            src = t
                        s1 = stp.tile([P, 1], f32, tag="s1")
                        nc.vector.reduce_sum(
                            out=s1[:], in_=src[:], axis=mybir.AxisListType.X
                        )
                        sq = wk.tile([P, HID], bf16, tag="posq")
                        s2 = stp.tile([P, 1], f32, tag="s2")
                        nc.scalar.activation(sq[:], src[:], AF.Square, accum_out=s2[:])
                        mean = stp.tile([P, 1], f32, tag="mean")
                        nc.vector.tensor_scalar_mul(mean[:], s1[:], 1.0 / HID)
                        var = stp.tile([P, 1], f32, tag="var")
                        nc.vector.tensor_scalar_mul(var[:], s2[:], 1.0 / HID)
                        msq = stp.tile([P, 1], f32, tag="msq")
                        nc.vector.tensor_tensor(
                            out=msq[:], in0=mean[:], in1=mean[:], op=ALU.mult
                        )
                        nc.vector.tensor_tensor(
                            out=var[:], in0=var[:], in1=msq[:], op=ALU.subtract
                        )
                        std = stp.tile([P, 1], f32, tag="std")
                        nc.scalar.activation(std[:], var[:], AF.Sqrt, bias=epst[:, 0:1])
                        rstd = stp.tile([P, 1], f32, tag="rstd")
                        nc.vector.reciprocal(rstd[:], std[:])
                        nmr = stp.tile([P, 1], f32, tag="nmr")
                        nc.vector.tensor_tensor(
                            out=nmr[:], in0=mean[:], in1=rstd[:], op=ALU.mult
                        )
                        nc.vector.tensor_scalar_mul(nmr[:], nmr[:], -1.0)

                        ocol = slice(b * HID, (b + 1) * HID)
                        xn = wk.tile([P, HID], f32, tag="poxn")
                        nc.scalar.activation(
                            xn[:], src[:], AF.Identity,
                            bias=nmr[:, 0:1], scale=rstd[:, 0:1],
                        )
                        if not fl["lng_one"]:
                            nc.vector.tensor_tensor(
                                out=xn[:], in0=xn[:], in1=lng_t[:], op=ALU.mult
                            )
                        if not fl["lnb_zero"]:
                            nc.vector.tensor_tensor(
                                out=xn[:], in0=xn[:], in1=lnb_t[:], op=ALU.add
                            )
                        if li == 0:
                            nc.vector.tensor_scalar_max(h_cur[:, ocol], xn[:], 0.0)
                        else:
                            r = wk.tile([P, HID], bf16, tag="por")
                            nc.vector.tensor_scalar_max(r[:], xn[:], 0.0)
                            # in-place residual: h_cur += relu(...)
                            nc.vector.tensor_add(h_cur[:, ocol], h_cur[:, ocol], r[:])
                            if li == NL - 1:
                                nc.vector.tensor_add(
                                    h_cur[:, ocol], h_cur[:, ocol], h0_sb[:, ocol]
                                )
                                nc.vector.tensor_scalar_add(
                                    h_cur[:, ocol], h_cur[:, ocol],
                                    mask_t[:, b : b + 1],
                                )

                        if li < NL - 1:
                            h8t = wk.tile([P, HID], f8, tag="po_h8")
                            nc.scalar.copy(h8t[:], h_cur[:, ocol])
                            nc.sync.dma_start(
                                out=h8[li + 1][b * P : (b + 1) * P, :], in_=h8t[:]
                            )
                        else:
                            nc.sync.dma_start(
                                out=h_fin[b * P : (b + 1) * P, :], in_=h_cur[:, ocol]
                            )

            # ================= pooling + MLP =================
            if stage < 7:
                with tc.tile_pool(name="stub", bufs=1) as stub:
                    zt = stub.tile([NCLS, GPC], f32, tag="zt")
                    nc.vector.memset(zt[:], 0.0)
                    nc.sync.dma_start(out=out[:], in_=zt[:])
            else:
              with tc.tile_pool(name="pool", bufs=2) as plp:
                # mean: one-hot matmul over SBUF-resident h, PSUM accumulate
                ppool_t = plp.tile([P, nnb * GPC], bf16, tag="Ppm")
                nc.sync.dma_start(out=ppool_t[:], in_=P_pm[:])
                mean_ps = [
                    pacc.tile([P, GPC], f32, tag=f"mean_ps{fc}", name=f"mean_ps{fc}")
                    for fc in range(4)
                ]
                for b in range(nnb):
                    for fc in range(4):
                        nc.tensor.matmul(
                            out=mean_ps[fc][:],
                            lhsT=h_cur[:, b * HID + fc * P : b * HID + (fc + 1) * P],
                            rhs=ppool_t[:, b * GPC : (b + 1) * GPC],
                            start=(b == 0),
                            stop=(b == nnb - 1),
                        )
                # max: per-graph transpose loads + reduce over the graph slot
                gmask_t = plp.tile([P, GPC], f32, tag="gmask")
                nc.sync.dma_start(out=gmask_t[:], in_=gmask[:])

                gkt = []  # 8 k-tiles of gT: 4 mean + 4 max, each [P, GPC] bf16
                for fc in range(4):
                    t = plp.tile([P, GPC], bf16, tag=f"gmean{fc}")
                    nc.vector.tensor_copy(out=t[:], in_=mean_ps[fc][:])
                    gkt.append(t)
                gslot = sched["gslot"]
                for fc in range(4):
                    mx = plp.tile([P, GPC], f32, tag=f"mx{fc}")
                    for g in range(GPC):
                        hT = plp.tile([P, gslot], bf16, tag="hT")
                        nc.sync.dma_start(
                            out=hT[:],
                            in_=h_fin[g * gslot : (g + 1) * gslot,
                                      fc * P : (fc + 1) * P],
                            transpose=True,
                        )
                        nc.vector.reduce_max(
                            out=mx[:, g : g + 1],
                            in_=hT[:],
                            axis=mybir.AxisListType.X,
                        )
                    mxm = plp.tile([P, GPC], bf16, tag=f"mxm{fc}")
                    nc.vector.tensor_tensor(
                        out=mxm[:], in0=mx[:], in1=gmask_t[:], op=ALU.mult
                    )
                    gkt.append(mxm)

                # MLP (transposed): a0T = relu(Wp0^T gT + b)
                wp0t = []
                for kt in range(8):
                    t = plp.tile([P, HID], bf16, tag=f"wp0_{kt}")
                    nc.sync.dma_start(out=t[:], in_=Wp0[kt * P : (kt + 1) * P, :])
                    wp0t.append(t)
                bp0_t = plp.tile([P, 4], f32, tag="bp0")
                nc.sync.dma_start(out=bp0_t[:], in_=bp0T[:])
                a0 = []
                for mt in range(4):
                    ps = pmlp.tile([P, GPC], f32, tag="mlp")
                    for kt in range(8):
                        nc.tensor.matmul(
                            out=ps[:],
                            lhsT=wp0t[kt][:, mt * P : (mt + 1) * P],
                            rhs=gkt[kt][:],
                            start=(kt == 0),
                            stop=(kt == 7),
                        )
                    t = plp.tile([P, GPC], bf16, tag=f"a0_{mt}")
                    nc.scalar.activation(
                        t[:], ps[:], AF.Relu, bias=bp0_t[:, mt : mt + 1]
                    )
                    a0.append(t)
                wp1t = []
                for kt in range(4):
                    t = plp.tile([P, HID // 2], bf16, tag=f"wp1_{kt}")
                    nc.sync.dma_start(out=t[:], in_=Wp1[kt * P : (kt + 1) * P, :])
                    wp1t.append(t)
                bp1_t = plp.tile([P, 2], f32, tag="bp1")
                nc.sync.dma_start(out=bp1_t[:], in_=bp1T[:])
                a1 = []
                for mt in range(2):
                    ps = pmlp.tile([P, GPC], f32, tag="mlp")
                    for kt in range(4):
                        nc.tensor.matmul(
                            out=ps[:],
                            lhsT=wp1t[kt][:, mt * P : (mt + 1) * P],
                            rhs=a0[kt][:],
                            start=(kt == 0),
                            stop=(kt == 3),
                        )
                    t = plp.tile([P, GPC], bf16, tag=f"a1_{mt}")
                    nc.scalar.activation(
                        t[:], ps[:], AF.Identity, bias=bp1_t[:, mt : mt + 1]
                    )
                    a1.append(t)
                wc0t = []
                for kt in range(2):
                    t = plp.tile([P, HID // 4], bf16, tag=f"wc0_{kt}")
                    nc.sync.dma_start(out=t[:], in_=Wc0[kt * P : (kt + 1) * P, :])
                    wc0t.append(t)
                bc0_t = plp.tile([P, 1], f32, tag="bc0")
                nc.sync.dma_start(out=bc0_t[:], in_=bc0T[:])
                ps = pmlp.tile([P, GPC], f32, tag="mlp")
                for kt in range(2):
                    nc.tensor.matmul(
                        out=ps[:],
                        lhsT=wc0t[kt][:],
                        rhs=a1[kt][:],
                        start=(kt == 0),
                        stop=(kt == 1),
                    )
                a2 = plp.tile([P, GPC], bf16, tag="a2")
                nc.scalar.activation(a2[:], ps[:], AF.Relu, bias=bc0_t[:, 0:1])
                wc1t = plp.tile([P, NCLS], bf16, tag="wc1")
                nc.sync.dma_start(out=wc1t[:], in_=Wc1[:])
                bc1_t = plp.tile([NCLS, 1], f32, tag="bc1")
                nc.sync.dma_start(out=bc1_t[:], in_=bc1[:])
                ps2 = pmlp.tile([NCLS, GPC], f32, tag="mlp")
                nc.tensor.matmul(
                    out=ps2[:], lhsT=wc1t[:], rhs=a2[:], start=True, stop=True
                )
                ot = plp.tile([NCLS, GPC], f32, tag="ot")
                nc.scalar.activation(ot[:], ps2[:], AF.Identity, bias=bc1_t[:, 0:1])
                nc.sync.dma_start(out=out[:], in_=ot[:])

    nc.compile()
    return nc


def make_in_maps(shared, per_core):
    maps = []
    for c in range(len(per_core)):
        m = dict(shared)
        m.update(per_core[c])
        m = {k: np.ascontiguousarray(v) for k, v in m.items()}
        maps.append(m)
    return maps


def kernel(**inputs) -> np.ndarray:
    sched, shared, per_core = preprocess(inputs)
    nc = build(sched, NCORES)
    in_maps = make_in_maps(shared, per_core)
    res = run_bass_kernel_spmd(nc, in_maps, list(range(NCORES)))
    full = np.zeros((NG, NCLS), np.float32)
    for c in range(NCORES):
        full[c * GPC : (c + 1) * GPC, :] = res.results[c]["out"].T
    return full
